# revision 51
# baseline (speedup 1.0000x reference)
"""GF(2) linear block encoder c = (b @ G) mod 2 on 8 TRN2 NeuronCores.

Strategy:
  - Data-parallel: shard b rows (32768 -> 8 x 4096), replicate G.
  - Bits {0,1} are exact in fp8-e4m3 and products accumulate exactly in
    fp32 PSUM, so the GF(2) matmul is an fp8 DoubleRow matmul (K=256 per
    MM). HW floor: 216ns per 512-col DR matmul (1 col/cycle @2.4GHz),
    512 MMs/core = 110.7us of PE streaming.
  - Extraction: ACT casts PSUM fp32 -> uint16, DVE ands with 1 and
    casts to uint8 rows staged in SBUF; host upcasts to int32.
  - DMA reality (measured): queues process ~25-50 descriptors/us each
    (contended across all 8 cores), one descriptor per partition per
    piece; descriptors up to ~8KB carry more bytes for the same count.
    Queue starts: sync ~8.2us, scalar ~8.6, gpsimd ~10.0.
  - Head: the ~640-descriptor critical set (b chunks 0-1 + all four
    kp h0 G pieces) is partition-split into 64-descriptor pieces across
    the three queues in exact consumption order; a kp-outer block over
    m-tiles 0-2 consumes G pieces as they land. Zeroed 512-col warmup
    matmuls hold the PE p-state at 2.4GHz through the supply window.
  - Output: m-tiles 0-9 run per-phase (G h1 hasn't landed yet) and ship
    1KB-descriptor half rows; m-tiles 10-31 run BOTH phases fused and
    ship one full 2KB-descriptor row each - descriptor demand stays
    ~37/us instead of crunching 4096 descriptors into phase 1. The last
    two rows leave partition-split so no queue holds a 128-descriptor
    piece at program end.
"""

import sys

import numpy as np

if "/opt/trn_rl_repo" not in sys.path:
    sys.path.insert(0, "/opt/trn_rl_repo")

import ml_dtypes

B_ROWS = 32768
K_MSG = 1024
N_CODE = 2048
NCORES = 8
M = B_ROWS // NCORES  # 4096 rows per core
KS = K_MSG // 128     # 8 k-subtiles of 128
KP = KS // 2          # 4 DoubleRow k-pair steps (K=256 each)
MT = M // 128         # 32 m-tiles
MC = 16               # b chunks along m (2 m-tiles each)
MCW = M // MC         # 256 rows per chunk
BG = 4                # b chunks per group tile
NBG = MC // BG        # 4 groups
HMT = 3               # head-block m-tiles (kp-outer, 3 PSUM half-tiles)
SPLIT_MT = 10         # m-tiles processed per-phase before fusing

F8 = ml_dtypes.float8_e4m3

_NC_CACHE = None


def _build_bass():
    import concourse.bacc as bacc
    import concourse.mybir as mybir
    from concourse import tile

    nc = bacc.Bacc("TRN2", target_bir_lowering=False, debug=False)

    # bt[p, c, s, j] = b bit for row m = c*MCW + j, k = s*128 + p
    bt = nc.dram_tensor("bt", [128, MC, KS, MCW], mybir.dt.float8e4, kind="ExternalInput")
    # g[p, kp, h, r, j] = G bit for k = (2*kp + r)*128 + p, n = h*1024 + j
    g = nc.dram_tensor("g", [128, KP, 2, 2, 1024], mybir.dt.float8e4, kind="ExternalInput")
    c = nc.dram_tensor("c", [M, N_CODE], mybir.dt.uint8, kind="ExternalOutput")

    dr = mybir.MatmulPerfMode.DoubleRow
    NH = N_CODE // 2

    with tile.TileContext(nc) as tc:
        with (
            tc.tile_pool(name="persist", bufs=1) as persist,
            tc.tile_pool(name="psum", bufs=4, space="PSUM") as psum_pool,
            tc.tile_pool(name="mids", bufs=8) as mids,
        ):
            # g_tiles[kp][p, h, r, j]
            g_tiles = [
                persist.tile([128, 2, 2, 1024], mybir.dt.float8e4, name=f"gt{kp}", tag=f"g{kp}")
                for kp in range(KP)
            ]
            b_groups = [
                persist.tile([128, BG, KS, MCW], mybir.dt.float8e4, name=f"bg{i}", tag=f"bg{i}")
                for i in range(NBG)
            ]

            def gh_part(kp, h, p0, p1, eng):
                # partition range of one (kp, n-half) G piece (2KB descs)
                eng.dma_start(out=g_tiles[kp][p0:p1, h], in_=g[p0:p1, kp, h])

            def bpair_part(ch, p0, p1, eng):
                # partition range of a chunk-PAIR (4KB descriptors)
                gi, sl = ch // BG, ch % BG
                eng.dma_start(
                    out=b_groups[gi][p0:p1, sl : sl + 2],
                    in_=bt[p0:p1, ch : ch + 2],
                )

            # --- input pushes: strict consumption order, partition-split
            # for the critical set so each queue's early pieces are only
            # 64 descriptors deep; b rides as chunk-pairs (4KB descs);
            # G h1 pieces come right after the critical set (needed from
            # the phase-1 revisit of m-tiles 0-9, ~30us).
            # sync (starts ~8.2us)
            gh_part(0, 0, 0, 64, nc.sync)
            gh_part(0, 0, 64, 128, nc.sync)
            gh_part(1, 0, 64, 128, nc.sync)
            gh_part(2, 0, 0, 64, nc.sync)
            bpair_part(2, 0, 128, nc.sync)     # chunks 2-3 (mt4-7)
            gh_part(0, 1, 0, 128, nc.sync)
            bpair_part(6, 0, 128, nc.sync)     # chunks 6-7 (mt12-15)
            gh_part(2, 1, 0, 128, nc.sync)
            bpair_part(8, 0, 128, nc.sync)     # chunks 8-9 (mt16-19)
            bpair_part(12, 0, 128, nc.sync)    # chunks 12-13 (mt24-27)
            # scalar (starts ~8.6us; issues done ~14.5, free for ACTs)
            bpair_part(0, 0, 64, nc.scalar)    # chunks 0-1 (mt0-3)
            bpair_part(0, 64, 128, nc.scalar)
            gh_part(2, 0, 64, 128, nc.scalar)
            gh_part(3, 0, 64, 128, nc.scalar)
            # gpsimd (starts ~10.0us)
            gh_part(1, 0, 0, 64, nc.gpsimd)
            gh_part(3, 0, 0, 64, nc.gpsimd)
            bpair_part(4, 0, 128, nc.gpsimd)   # chunks 4-5 (mt8-11)
            gh_part(1, 1, 0, 128, nc.gpsimd)
            gh_part(3, 1, 0, 128, nc.gpsimd)
            bpair_part(10, 0, 128, nc.gpsimd)  # chunks 10-11 (mt20-23)
            bpair_part(14, 0, 128, nc.gpsimd)  # chunks 14-15 (mt28-31)

            # --- PE warmups on zeroed dummy tiles into a dedicated PSUM
            # tile: no data deps beyond the early DVE memsets, so they run
            # from ~7.1us and hold the DVFS ramp until real data lands.
            zw0 = persist.tile([128, 2, 128], mybir.dt.float8e4, name="zw0")
            zw = persist.tile([128, 2, 512], mybir.dt.float8e4, name="zwarm")
            nc.vector.memset(zw0, 0)
            nc.vector.memset(zw, 0)
            ps_warm = psum_pool.tile([128, NH], mybir.dt.float32, name="ps")

            def warm(cols=512):
                src = zw0 if cols <= 128 else zw
                nc.tensor.matmul(
                    ps_warm[:, 0:cols],
                    src[:, :, 0:128],
                    src[:, :, 0:cols],
                    start=True,
                    stop=True,
                    perf_mode=dr,
                )

            for _ in range(4):
                warm(64)
            for _ in range(7):
                warm(512)

            # output viewed per m-tile: m = mt*128 + p
            c_view = c.rearrange("(mt p) n -> mt p n", p=128)

            out_eng = [nc.gpsimd, nc.sync, nc.scalar]

            # full-row output staging: both n-halves of an m-tile leave as
            # ONE dma with 2KB-per-partition descriptors once fused.
            c8all = persist.tile([128, MT, N_CODE], mybir.dt.uint8, name="c8all")

            def bsta(mt, kp):
                mc, j = mt // 2, mt % 2
                return b_groups[mc // BG][
                    :, mc % BG, 2 * kp : 2 * kp + 2, j * 128 : (j + 1) * 128
                ]

            def grhs(kp, ph, q):
                # [128, 2, 512] moving operand: n-cols ph*1024+q*512 ..+512
                return g_tiles[kp][:, ph, :, q * 512 : (q + 1) * 512]

            def extract(mid, cdst, ps, m0, m1, o0, o1):
                # PSUM fp32 -> uint16 (ACT cast) -> &1 (DVE) -> uint8 row
                nc.scalar.activation(
                    mid[:, m0:m1], ps, mybir.ActivationFunctionType.Copy
                )
                nc.vector.tensor_scalar(
                    out=mid[:, m0:m1], in0=mid[:, m0:m1], scalar1=1,
                    scalar2=None, op0=mybir.AluOpType.bitwise_and,
                )
                nc.vector.tensor_scalar(
                    out=cdst[:, o0:o1], in0=mid[:, m0:m1], scalar1=0,
                    scalar2=None, op0=mybir.AluOpType.bypass,
                )

            def mm_half(ps, mt, kp, ph):
                for nt in range(2):
                    nc.tensor.matmul(
                        ps[:, nt * 512 : (nt + 1) * 512],
                        bsta(mt, kp),
                        grhs(kp, ph, nt),
                        start=(kp == 0),
                        stop=(kp == KP - 1),
                        perf_mode=dr,
                    )

            def ship_half(mt, ph, eng):
                eng.dma_start(
                    out=c_view[mt][:, ph * NH : (ph + 1) * NH],
                    in_=c8all[:, mt, ph * NH : (ph + 1) * NH],
                )

            # --- phase-0 head block: m-tiles 0..2, kp-OUTER so each G h0
            # piece is consumed the moment it lands; seam warmups after the
            # first two kp rounds absorb arrival jitter.
            head_ps = [
                psum_pool.tile([128, NH], mybir.dt.float32, name="ps")
                for _ in range(HMT)
            ]
            for kp in range(KP):
                for hm in range(HMT):
                    mm_half(head_ps[hm], hm, kp, 0)
                if kp < 2:
                    warm(512)
            for hm in range(HMT):
                mid = mids.tile([128, NH], mybir.dt.uint16)
                extract(mid, c8all[:, hm], head_ps[hm], 0, NH, 0, NH)

            def do_half(mt, ph, ship):
                ps = psum_pool.tile([128, NH], mybir.dt.float32, name="ps")
                for kp in range(KP):
                    mm_half(ps, mt, kp, ph)
                mid = mids.tile([128, NH], mybir.dt.uint16)
                n0 = ph * NH
                extract(mid, c8all[:, mt], ps, 0, NH, n0, n0 + NH)
                if ship:
                    ship_half(mt, ph, out_eng[(ph * MT + mt) % 3])

            # --- per-phase stretch while G h1 is still arriving. No output
            # dmas yet (they'd fight the input descriptors): rows stage in
            # c8all and drip out during the fused stretch.
            for mt in range(HMT, SPLIT_MT):
                do_half(mt, 0, False)
            for mt in range(SPLIT_MT):
                do_half(mt, 1, False)

            # --- fused stretch: both halves back-to-back, one full-row dma
            # per tile plus one deferred early row every other tile. The
            # scalar queue's descriptor generation is starved while ACT
            # runs, so it gets only 1 row in 5.
            row_seq = [nc.gpsimd, nc.sync, nc.gpsimd, nc.sync, nc.scalar]
            row_i = [0]

            def row_eng():
                e = row_seq[row_i[0] % len(row_seq)]
                row_i[0] += 1
                return e

            deferred = list(range(SPLIT_MT))
            for mt in range(SPLIT_MT, MT):
                last2 = mt >= MT - 2
                do_half(mt, 0, False)
                if mt == MT - 1:
                    # very last tile: its phase-0 half leaves the moment it
                    # extracted, partition-split three ways
                    for i, eng in enumerate([nc.sync, nc.scalar, nc.gpsimd]):
                        p0, p1 = [(0, 43), (43, 86), (86, 128)][i]
                        eng.dma_start(
                            out=c_view[mt][p0:p1, 0:NH], in_=c8all[p0:p1, mt, 0:NH]
                        )
                if not last2:
                    do_half(mt, 1, False)
                else:
                    # per-bank PSUM quarters for the final half so the tail
                    # is one 512-col extract chain
                    mid = mids.tile([128, NH], mybir.dt.uint16)
                    for nt in range(2):
                        psq = psum_pool.tile([128, 512], mybir.dt.float32, name="ps")
                        for kp in range(KP):
                            nc.tensor.matmul(
                                psq,
                                bsta(mt, kp),
                                grhs(kp, 1, nt),
                                start=(kp == 0),
                                stop=(kp == KP - 1),
                                perf_mode=dr,
                            )
                        m0, m1 = nt * 512, (nt + 1) * 512
                        extract(mid, c8all[:, mt], psq, m0, m1, NH + m0, NH + m1)
                if mt == MT - 2:
                    nc.gpsimd.dma_start(out=c_view[mt][0:64], in_=c8all[0:64, mt])
                    nc.sync.dma_start(out=c_view[mt][64:128], in_=c8all[64:128, mt])
                elif mt == MT - 1:
                    # very last row: six 43-descriptor pieces, two per queue,
                    # the phase-0 half as soon as it extracted
                    for i, eng in enumerate([nc.scalar, nc.gpsimd, nc.sync]):
                        p0, p1 = [(0, 43), (43, 86), (86, 128)][i]
                        eng.dma_start(
                            out=c_view[mt][p0:p1, NH:], in_=c8all[p0:p1, mt, NH:]
                        )
                else:
                    row_eng().dma_start(out=c_view[mt], in_=c8all[:, mt])
                    if mt % 2 == 0 and deferred:
                        emt = deferred.pop(0)
                        row_eng().dma_start(out=c_view[emt], in_=c8all[:, emt])


    nc.finalize()
    return nc


def _get_nc():
    global _NC_CACHE
    if _NC_CACHE is None:
        _NC_CACHE = _build_bass()
    return _NC_CACHE


def _pack_inputs(b, G):
    b8 = np.asarray(b).astype(np.uint8)
    G8 = np.asarray(G).astype(np.uint8)
    # g[p, kp, h, r, j]: k = (2*kp + r)*128 + p, n = h*1024 + j
    g_psn = G8.reshape(KS, 128, N_CODE).transpose(1, 0, 2)   # [p, s, n]
    g_f8 = (
        g_psn.reshape(128, KP, 2, 2, 1024)                    # [p, kp, r, h, j]
        .transpose(0, 1, 3, 2, 4)                             # [p, kp, h, r, j]
        .astype(F8, order="C")
    )
    bts = []
    for core in range(NCORES):
        sh = b8[core * M : (core + 1) * M]  # [M, K]
        # bt[p, c, s, j]: m = c*MCW + j, k = s*128 + p
        btc = sh.reshape(MC, MCW, KS, 128).transpose(3, 0, 2, 1)
        bts.append(btc.astype(F8, order="C"))
    return bts, g_f8


def kernel(b, G, trace=False, **run_kwargs):
    from concourse.bass_utils import run_bass_kernel_spmd

    nc = _get_nc()
    bts, g_f8 = _pack_inputs(b, G)
    in_maps = [{"bt": bts[i], "g": g_f8} for i in range(NCORES)]
    res = run_bass_kernel_spmd(
        nc, in_maps, core_ids=list(range(NCORES)), trace=trace, **run_kwargs
    )
    out = np.concatenate([res.results[i]["c"] for i in range(NCORES)], axis=0)
    out = out.astype(np.int32)
    if trace:
        kernel.last_results = res
    return out


kernel.last_results = None
